# revision 64
# baseline (speedup 1.0000x reference)
"""Trainium2 Bass kernel for a cross-attention block.

Reference computation (per batch b of 2):
  qc   = conv3x3(q)                      # [256, 64, 64], SAME padding
  qn   = rmsnorm(qc, over channel) * g_q
  kn   = rmsnorm(k,  over channel) * g_k
  qp   = qn @ wq.T  (bq == 0)            # [4096, 256] -> 8 heads x 32
  kp   = kn @ wk.T  (bk == 0)            # [1024, 256]
  s    = qp . kp / sqrt(32) per head, masked to a local window
  attn = mean_h softmax_k(s)             # [4096, 1024]
  out  = attn @ v_flat                   # [4096, 256] -> [256, 64, 64]

Sharding: 8 cores = (batch 2) x (16-row query stripes 4). Each core computes
its stripe's conv (halo rows packed on the host), the k/v projections for the
14 key rows its queries can see, and windowed masked attention.

Implementation notes (cost-model driven):
  - matmul cost is charged at SEQ-issue time with a p-state ramp; a run of
    tiny warm-up matmuls up front keeps every real matmul at full clock.
  - the window mask is folded into the score PSUM as an additive bias via an
    identity-matmul pre-init per bank (PE), eliminating the DVE mask multiply.
    PSUM is bank-granular and one bank tolerates only one tile_position, so
    scores go out as four 2-head groups (one head per bank, bufs=3 rotation).
  - one wide exp Activation per 2-head group (strided across its banks) with
    the q-side rmsnorm folded in as the per-partition scale.
  - per-query mean-square is computed directly transposed: s-major squares as
    the stationary operand x ones -> out free size 1 (nearly free on PE);
    rinvq via a DVE Newton chain, keeping Ln off the pre-loop Act path (one
    act-table set switch pair stays hidden under the conv).
  - the k-side chain (msk/ln/exp/rbk/kn/pk) is interleaved between conv
    quarters so each cross-engine latency hides under conv matmuls.
  - softmax: DVE half-add + reduce + reciprocal; heads 0-3 scaled+folded on
    the Pool engine (broadcast multiply), heads 4-7 via a fused DVE
    scalar_tensor_tensor chain. Work spread over all four compute engines.
  - v resident in SBUF as window pieces; merged attn transposed in 2 PE
    transposes so feat accumulates straight from vres; both feat halves share
    one PSUM bank so the outbuf copy is a single Act op; 3 output staging
    tiles so output DMAs never WAR-block later tiles.
"""

import os
from contextlib import ExitStack

import numpy as np

import concourse.bacc as bacc
import concourse.bass as bass
import concourse.tile as tile
from concourse import mybir
from concourse.bass_utils import run_bass_kernel_spmd

F32 = mybir.dt.float32
F16 = mybir.dt.float16
AF = mybir.ActivationFunctionType
ALU = mybir.AluOpType

B, C, H, W = 2, 256, 64, 64
HK, WK = 32, 32
NH, HD = 8, 32
EPS = 1e-6
SCALE = 1.0 / np.sqrt(HD)
MASK_BIAS = -20000.0

NCORES = 8
RSTRIPE = 16            # query rows per core
KI = 14                 # key rows per core window
KJ = 10                 # key cols per q-tile window
NW = KJ * KI            # 140 keys per q-tile window
NT = 8                  # q-tiles per core (16y x 8x each)
KJ0 = [max(0, min(4 * s - 3, WK - KJ)) for s in range(NT)]
KI0 = [max(0, min(8 * r - 3, HK - KI)) for r in range(4)]

NWARM = int(os.environ.get("NWARM", "62"))
WARMF = int(os.environ.get("WARMF", "64"))


def build_nc():
    nc = bacc.Bacc()
    qpad_d = [
        nc.declare_dram_parameter(f"qpad{i}", [128, 18, 66], F16, isOutput=False)
        for i in range(2)
    ]
    wt_d = [
        [
            nc.declare_dram_parameter(f"wt{ci}{co}", [128, 9, 128], F16, isOutput=False)
            for co in range(2)
        ]
        for ci in range(2)
    ]
    # params blob columns: wqt (2x256) | wkt (2x256) | kin (2x448) | mskb (8x140) | ident (128)
    PCOLS = 512 + 512 + 896 + NT * NW + 128
    par_d = nc.declare_dram_parameter("par", [128, PCOLS], F16, isOutput=False)
    vin_d = nc.declare_dram_parameter("vin", [70, 2 * NT, 256], F16, isOutput=False)
    out_d = nc.declare_dram_parameter("out", [2, 128, NT, 128], F32, isOutput=True)

    with tile.TileContext(nc) as tc, ExitStack() as ctx:
        sing = ctx.enter_context(tc.tile_pool(name="sing", bufs=1))

        # ---- constants for warm-up (must be ready almost immediately) ----
        wz = sing.tile([128, WARMF], F16, name="wz")
        nc.gpsimd.memset(wz[:], 0.0)
        ones_col = sing.tile([128, 1], F16, name="onesc")
        nc.vector.memset(ones_col[:], 1.0)
        ones_row = sing.tile([1, 128], F16, name="onesr")
        nc.vector.memset(ones_row[:], 1.0)
        eps1 = sing.tile([1, 1], F32, name="eps1")
        nc.vector.memset(eps1[:], EPS)

        # ---- input DMAs, conv-critical order ----
        qpad_t = [sing.tile([128, 18, 66], F16, name=f"qpad{i}") for i in range(2)]
        wt_t = [[sing.tile([128, 9, 128], F16, name=f"wt{ci}{co}") for co in range(2)]
                for ci in range(2)]
        par_t = sing.tile([128, PCOLS], F16, name="par")
        vres = sing.tile([70, 2 * NT, 256], F16, name="vres")
        # wt00 + the first 11 qpad0 rows unblock conv (ci0, n2=0) earliest
        nc.sync.dma_start(wt_t[0][0][:], wt_d[0][0][:])
        nc.sync.dma_start(qpad_t[0][:, 0:11, :], qpad_d[0][:, 0:11, :])
        nc.sync.dma_start(qpad_t[0][:, 11:18, :], qpad_d[0][:, 11:18, :])
        nc.sync.dma_start(qpad_t[1][:], qpad_d[1][:])
        nc.sync.dma_start(wt_t[1][0][:], wt_d[1][0][:])
        nc.sync.dma_start(par_t[:], par_d[:])
        nc.sync.dma_start(wt_t[0][1][:], wt_d[0][1][:])
        nc.sync.dma_start(wt_t[1][1][:], wt_d[1][1][:])
        nc.sync.dma_start(vres[:], vin_d[:])

        wqt_v = par_t[:, 0:512].rearrange("p (t c) -> p t c", t=2)
        wkt_v = par_t[:, 512:1024].rearrange("p (t c) -> p t c", t=2)
        kin_v = par_t[:, 1024:1920].rearrange("p (t c) -> p t c", t=2)
        msk_v = par_t[:, 1920 : 1920 + NT * NW].rearrange("p (s w) -> p s w", s=NT)
        ident_v = par_t[:, 1920 + NT * NW : 1920 + NT * NW + 128]

        qcT = [sing.tile([128, 1024], F16, name=f"qcT{i}") for i in range(2)]
        sqs = [sing.tile([128, NT, 128], F16, name=f"sqs{i}") for i in range(2)]
        sqk = sing.tile([128, 2, 448], F16, name="sqk")
        kn = sing.tile([128, 2, 448], F16, name="kn")
        qpT = [sing.tile([128, NT, 128], F16, name=f"qpT{i}") for i in range(2)]
        kpT = [sing.tile([128, 448], F16, name=f"kpT{i}") for i in range(2)]
        rinvq = sing.tile([128, NT], F32, name="rinvq")
        lnk = sing.tile([1, 448], F32, name="lnk")
        rinvk = sing.tile([1, 448], F16, name="rinvk")
        # three staging tiles matching the three output DMA chunks, so a DMA
        # read never WAR-blocks later outcopies (tile-level dep tracking)
        outbuf = [
            sing.tile([128, 2, 4, 128], F32, name="outbufA"),
            sing.tile([128, 2, 2, 128], F32, name="outbufB"),
            sing.tile([128, 2, 1, 128], F32, name="outbufC"),
            sing.tile([128, 2, 1, 128], F32, name="outbufD"),
        ]
        OB = {0: (outbuf[0], 0), 1: (outbuf[0], 1), 2: (outbuf[0], 2), 3: (outbuf[0], 3),
              4: (outbuf[1], 0), 5: (outbuf[1], 1), 6: (outbuf[2], 0), 7: (outbuf[3], 0)}

        # ---- PE warm-up: keep the exec queue stuffed so every real matmul
        # is costed after the p-state ramp completes ----
        with tc.tile_pool(name="ps_warm", bufs=1, space="PSUM") as warm:
            wps = warm.tile([1, WARMF], F32, name="wps", tag="wps")
            for _ in range(NWARM):
                nc.tensor.matmul(wps[:], ones_col[:], wz[:], start=True, stop=True)

        # ---- k-side squares (DVE, overlaps conv) ----
        for ct in range(2):
            nc.vector.tensor_mul(sqk[:, ct, :], kin_v[:, ct, :], kin_v[:, ct, :])

        with tc.tile_pool(name="ps_pre", bufs=1, space="PSUM") as pre:
            cv = [[None, None], [None, None]]
            for co_t in range(2):
                for n2 in range(2):
                    cv[co_t][n2] = pre.tile(
                        [128, 512], F32, name=f"cv{co_t}{n2}", tag=f"cv{co_t}{n2}"
                    )

            def conv_quarter(co_t, n2, emit_copy=True):
                for ci in range(2):
                    for tap in range(9):
                        dy, dx = divmod(tap, 3)
                        lhsT = wt_t[ci][co_t][:, tap, :]
                        rhs = qpad_t[ci][:, dy + 8 * n2 : dy + 8 * n2 + 8, dx : dx + 64]
                        nc.tensor.matmul(
                            cv[co_t][n2][:],
                            lhsT,
                            rhs,
                            start=(ci == 0 and tap == 0),
                            stop=(ci == 1 and tap == 8),
                        )
                if emit_copy:
                    quarter_copy(co_t, n2)

            def quarter_copy(co_t, n2):
                sl = slice(512 * n2, 512 * (n2 + 1))
                nc.scalar.copy(qcT[co_t][:, sl], cv[co_t][n2][:])
                # squares written s-major: sqs[ct][:, s, y*8+x] = qcT[y, 8s+x]^2
                sv = sqs[co_t][:].rearrange("p s (y x) -> p y s x", y=16)
                sv2 = sv[:, 8 * n2 : 8 * n2 + 8, :, :]
                qv = qcT[co_t][:, sl].rearrange("p (y s x) -> p y s x", s=8, x=8)
                nc.vector.tensor_mul(sv2, qv, qv)

            # k-side chain (msk -> lnk/rinvk -> rbk -> kn -> pk -> kpT) is
            # interleaved between conv quarters so each cross-engine latency
            # hides under ~3.8us of conv matmuls.
            msk_ps = pre.tile([1, 448], F32, name="mskps", tag="msk")
            rbk_ps = pre.tile([128, 448], F32, name="rbkps", tag="rbk")
            pk = [None, None]
            for co_t in range(2):
                pk[co_t] = pre.tile([128, 448], F32, name=f"pk{co_t}", tag=f"pk{co_t}")

            conv_quarter(0, 0, emit_copy=False)
            for ct in range(2):
                nc.tensor.matmul(
                    msk_ps[:], ones_col[:], sqk[:, ct, :], start=(ct == 0), stop=(ct == 1)
                )
            nc.scalar.activation(lnk[:], msk_ps[:], AF.Ln, bias=eps1[:], scale=1.0 / C)
            nc.scalar.activation(rinvk[:], lnk[:], AF.Exp, scale=-0.5)
            quarter_copy(0, 0)

            conv_quarter(0, 1)
            # rbk = broadcast rinv_k over partitions; kn = kin * rbk (DVE)
            nc.tensor.matmul(rbk_ps[:], ones_row[:], rinvk[:], start=True, stop=True)
            for ct in range(2):
                nc.vector.tensor_mul(kn[:, ct, :], kin_v[:, ct, :], rbk_ps[:])

            conv_quarter(1, 0)
            for co_t in range(2):
                for ct in range(2):
                    nc.tensor.matmul(
                        pk[co_t][:],
                        wkt_v[:, ct, 128 * co_t : 128 * (co_t + 1)],
                        kn[:, ct, :],
                        start=(ct == 0),
                        stop=(ct == 1),
                    )
            for co_t in range(2):
                nc.scalar.copy(kpT[co_t][:], pk[co_t][:])

            conv_quarter(1, 1)

        # ---- projections + transposed mean-square (all at full PE clock) ----
        with tc.tile_pool(name="ps_proj", bufs=1, space="PSUM") as psp:
            pq = [[None, None], [None, None]]
            for co_t in range(2):
                for n2 in range(2):
                    pq[co_t][n2] = psp.tile(
                        [128, 4, 128], F32, name=f"pq{co_t}{n2}", tag=f"pq{co_t}{n2}"
                    )
            rT = psp.tile([128, NT], F32, name="rT", tag="rT")

            # transposed mean-square first: out free size 1 per (s, ct), so the
            # DVE Newton chain for rinvq can overlap the projection matmuls
            for s in range(NT):
                for ct in range(2):
                    nc.tensor.matmul(
                        rT[:, s : s + 1],
                        sqs[ct][:, s, :],
                        ones_col[:],
                        start=(ct == 0),
                        stop=(ct == 1),
                    )
            # rinv_q = rsqrt(ms/C + eps): 2 Newton steps on DVE (keeps Ln off
            # the Act critical path -> no act-table switch before the loop)
            x_t = sing.tile([128, NT], F32, name="nx")
            y_t = sing.tile([128, NT], F32, name="ny")
            t_t = sing.tile([128, NT], F32, name="nt")
            u_t = sing.tile([128, NT], F32, name="nu")
            nc.vector.tensor_scalar(x_t[:], rT[:], 1.0 / C, EPS, ALU.mult, ALU.add)
            nc.vector.tensor_scalar(y_t[:], x_t[:], -0.527, 1.607, ALU.mult, ALU.add)
            for _ in range(3):
                nc.vector.tensor_mul(t_t[:], y_t[:], y_t[:])
                nc.vector.tensor_mul(t_t[:], t_t[:], x_t[:])
                nc.vector.tensor_scalar(u_t[:], t_t[:], -0.5, 1.5, ALU.mult, ALU.add)
                nc.vector.tensor_mul(y_t[:], y_t[:], u_t[:])
            nc.vector.tensor_copy(rinvq[:], y_t[:])

            # q projection: all ct=0 contributions first (qcT[0] is ready long
            # before qcT[1]), then ct=1 per co_t half followed by its copies.
            # One matmul per (co_t, n2, ct) bank: the 4 si blocks ride along as
            # extra free dims of the moving operand (one psum group per bank).
            for ct in range(2):
                for co_t in range(2):
                    for n2 in range(2):
                        rhs = qcT[ct][:].rearrange(
                            "p (y si x) -> p si y x", si=8, x=8
                        )[:, 4 * n2 : 4 * n2 + 4, :, :]
                        nc.tensor.matmul(
                            pq[co_t][n2][:].rearrange("p s q -> p (s q)"),
                            wqt_v[:, ct, 128 * co_t : 128 * (co_t + 1)],
                            rhs,
                            start=(ct == 0),
                            stop=(ct == 1),
                        )
                    if ct == 1:
                        # psum -> sbuf copies for this half, Act and DVE in parallel
                        qf = qpT[co_t][:].rearrange("p s q -> p (s q)")
                        pf0 = pq[co_t][0][:].rearrange("p s q -> p (s q)")
                        pf1 = pq[co_t][1][:].rearrange("p s q -> p (s q)")
                        nc.scalar.copy(qf[:, 0:512], pf0)
                        nc.vector.tensor_copy(qf[:, 512:1024], pf1)

        # ---- windowed masked attention, software-pipelined over 8 q-tiles ----
        att = ctx.enter_context(tc.tile_pool(name="att", bufs=1))
        with tc.tile_pool(name="ps_att", bufs=1, space="PSUM") as psa:
            e_state = {}
            state = {}

            def emit_scores_group(s, g):
                # group g covers heads 2g, 2g+1; half a = g // 2 selects qpT/kpT
                a, p = divmod(g, 2)
                kj0 = KJ0[s]
                sc = psa.tile([128, 2, 512], F32, name=f"sc{g}", tag="sc", bufs=3)
                for j in range(2):
                    hh = 2 * p + j  # 32-row block within half a
                    nc.tensor.matmul(
                        sc[:, j, 0:NW],
                        ident_v[:],
                        msk_v[:, s, :],
                        start=True,
                        stop=False,
                        skip_group_check=True,
                    )
                    nc.tensor.matmul(
                        sc[:, j, 0:NW],
                        qpT[a][32 * hh : 32 * hh + 32, s, :],
                        kpT[a][32 * hh : 32 * hh + 32, 14 * kj0 : 14 * kj0 + NW],
                        start=False,
                        stop=True,
                        tile_position=(32 * hh, 0),
                        skip_group_check=True,
                    )
                return sc

            def emit_exp_group(s, g, sc):
                if g == 0:
                    e_t = att.tile([128, NH, NW], F16, tag="e", bufs=3)
                    e_state[s] = e_t
                else:
                    e_t = e_state[s]
                nc.scalar.activation(
                    e_t[:, 2 * g : 2 * g + 2, :],
                    sc[:, :, 0:NW],
                    AF.Exp,
                    scale=rinvq[:, s : s + 1],
                )

            def emit_softmax(s):
                e_t = e_state.pop(s)
                eh = att.tile([128, NH, 70], F16, tag="eh", bufs=3)
                sums = att.tile([128, NH], F32, tag="sums", bufs=3)
                rs = att.tile([128, NH], F32, tag="rs", bufs=3)
                scl = att.tile([128, 4, NW], F16, tag="scl", bufs=3)
                s47 = att.tile([128, 4, NW], F16, tag="s47", bufs=3)
                attn = att.tile([128, NW], F16, tag="attnm", bufs=3)
                nc.vector.tensor_add(eh[:], e_t[:, :, 0:70], e_t[:, :, 70:140])
                nc.vector.reduce_sum(out=sums[:], in_=eh[:], axis=mybir.AxisListType.X)
                nc.vector.reciprocal(rs[:], sums[:])
                if s < NT - 2:
                    # steady state: heads 0-3 scale+fold on Pool (throughput)
                    rs16 = att.tile([128, 4], F16, tag="rs16", bufs=3)
                    t03 = att.tile([128, 2, NW], F16, tag="t03", bufs=3)
                    a03 = att.tile([128, NW], F16, tag="a03", bufs=3)
                    with nc.allow_low_precision(reason="1/sum feeds f16 attn scale"):
                        nc.vector.tensor_copy(rs16[:], rs[:, 0:4])
                    nc.gpsimd.tensor_mul(
                        scl[:], e_t[:, 0:4, :], rs16[:, :, None].broadcast_to((128, 4, NW))
                    )
                    nc.gpsimd.tensor_add(t03[:], scl[:, 0:2, :], scl[:, 2:4, :])
                    nc.gpsimd.tensor_add(a03[:], t03[:, 0, :], t03[:, 1, :])
                else:
                    # pipeline drain: the slow serial Pool chain would sit on
                    # the critical path; run heads 0-3 as a DVE chain instead
                    nc.vector.tensor_scalar_mul(scl[:, 0, :], e_t[:, 0, :], rs[:, 0:1])
                    for j in range(3):
                        nc.vector.scalar_tensor_tensor(
                            out=scl[:, j + 1, :],
                            in0=e_t[:, 1 + j, :],
                            scalar=rs[:, 1 + j : 2 + j],
                            in1=scl[:, j, :],
                            op0=ALU.mult,
                            op1=ALU.add,
                        )
                    a03 = scl[:, 3, :]
                # heads 4-7: fused scale+add chain on DVE
                nc.vector.tensor_scalar_mul(s47[:, 0, :], e_t[:, 4, :], rs[:, 4:5])
                for j in range(3):
                    nc.vector.scalar_tensor_tensor(
                        out=s47[:, j + 1, :],
                        in0=e_t[:, 5 + j, :],
                        scalar=rs[:, 5 + j : 6 + j],
                        in1=s47[:, j, :],
                        op0=ALU.mult,
                        op1=ALU.add,
                    )
                if s < NT - 2:
                    nc.vector.tensor_add(attn[:], a03[:], s47[:, 3, :])
                else:
                    nc.vector.tensor_add(attn[:], scl[:, 3, :], s47[:, 3, :])
                state[s] = attn

            def emit_tail(s):
                attn = state.pop(s)
                at_sb = []
                for g in range(2):
                    at_p = psa.tile([70, 128], F16, name=f"at{g}", tag="at")
                    nc.tensor.transpose(
                        at_p[:], attn[:, 70 * g : 70 * (g + 1)], ident_v[:]
                    )
                    t = att.tile([70, 128], F16, tag=f"atT{g}", bufs=3)
                    nc.vector.tensor_copy(t[:], at_p[:])
                    at_sb.append(t)
                # both co_t halves in one bank (same tile_position, sequential
                # groups) so the psum->outbuf copy is a single Act instruction
                ft_p = psa.tile([128, 2, 128], F32, name="ft", tag="ft")
                for co_t in range(2):
                    for g in range(2):
                        nc.tensor.matmul(
                            ft_p[:, co_t, :],
                            vres[:, 2 * s + g, 128 * co_t : 128 * (co_t + 1)],
                            at_sb[g][:],
                            start=(g == 0),
                            stop=(g == 1),
                        )
                ob, off = OB[s]
                nc.scalar.copy(ob[:, :, off, :], ft_p[:])

            def out_dma(chunk):
                lo, hi = [(0, 4), (4, 6), (6, 7), (7, 8)][chunk]
                # dst AP reordered to match the src (p, co, s, x) iteration
                dst = out_d[:].rearrange("co p s x -> p co s x")[:, :, lo:hi, :]
                nc.sync.dma_start(dst, outbuf[chunk][:])

            for s in range(NT):
                for g in range(4):
                    sc = emit_scores_group(s, g)
                    emit_exp_group(s, g, sc)
                if s >= 1:
                    emit_tail(s - 1)
                    if s == 4:
                        out_dma(0)
                    if s == 6:
                        out_dma(1)
                    if s == 7:
                        out_dma(2)
                emit_softmax(s)
            emit_tail(NT - 1)
            out_dma(3)
    nc.compile()
    return nc


def _host_prep(q, k, v, conv_w, g_q, g_k, wq, bq, wk, bk):
    f = np.float32
    h = np.float16
    assert np.abs(bq).max() == 0.0 and np.abs(bk).max() == 0.0, "kernel folds biases=0"
    q = np.ascontiguousarray(q, dtype=f)
    k = np.ascontiguousarray(k, dtype=f)
    v = np.ascontiguousarray(v, dtype=f)
    wt = (
        np.ascontiguousarray(conv_w, dtype=f)
        .transpose(2, 3, 1, 0)
        .reshape(9, 2, 128, 256)
        .transpose(1, 2, 0, 3)
    )  # [ci_t, 128, 9, co 256]
    wt4 = [
        [np.ascontiguousarray(wt[ci, :, :, 128 * co : 128 * (co + 1)], dtype=h) for co in range(2)]
        for ci in range(2)
    ]
    wqt = (wq.T * g_q[:, None] * SCALE).reshape(2, 128, 256)  # [ct, ch, co]
    wkt = (wk.T * g_k[:, None]).reshape(2, 128, 256)
    wqt_p = np.ascontiguousarray(wqt.transpose(1, 0, 2).reshape(128, 512), dtype=h)
    wkt_p = np.ascontiguousarray(wkt.transpose(1, 0, 2).reshape(128, 512), dtype=h)
    ident = np.eye(128, dtype=h)

    # additive mask bias per stripe r: [NT, 128, NW], 0 allowed / MASK_BIAS blocked
    masks = []
    for r in range(4):
        ki = KI0[r] + np.arange(KI, dtype=f)
        m_r = np.empty((NT, 128, NW), dtype=f)
        y = 16 * r + np.arange(RSTRIPE, dtype=f)
        ci = (y + 0.5) * 0.5 - 0.5
        oki = np.abs(ci[:, None] - ki[None, :]) <= 3.0  # [16, 14]
        for s in range(NT):
            kj = KJ0[s] + np.arange(KJ, dtype=f)
            x = 8 * s + np.arange(8, dtype=f)
            cj = (x + 0.5) * 0.5 - 0.5
            okj = np.abs(cj[:, None] - kj[None, :]) <= 3.0  # [8, 10]
            m = oki[:, None, None, :] & okj[None, :, :, None]  # [yl, xl, kjl, kil]
            m_r[s] = np.where(m.reshape(128, NW), 0.0, MASK_BIAS)
        masks.append(np.ascontiguousarray(m_r.transpose(1, 0, 2).reshape(128, NT * NW), dtype=h))

    in_maps = []
    for core in range(NCORES):
        b, r = divmod(core, 4)
        qpad = np.zeros((256, 18, 66), dtype=f)
        lo = max(0, 16 * r - 1)
        hi = min(64, 16 * r + 17)
        qpad[:, lo - (16 * r - 1) : hi - (16 * r - 1), 1:65] = q[b, :, lo:hi, :]
        qpad = qpad.reshape(2, 128, 18, 66).astype(h)
        ki0 = KI0[r]
        ksl = k[b][:, ki0 : ki0 + KI, :]  # [256, 14, 32]
        kin = ksl.transpose(0, 2, 1).reshape(2, 128, 448)  # [ct, ch, w=kj*14+ki]
        kin_p = np.ascontiguousarray(kin.transpose(1, 0, 2).reshape(128, 896), dtype=h)
        # 1/NH folds the mean-over-heads into the value matmul
        vin = (
            v[b][:, ki0 : ki0 + KI, :].transpose(2, 1, 0).reshape(448, 256) / NH
        )  # [w, d]
        vin_p = np.empty((70, 2 * NT, 256), dtype=h)
        for s in range(NT):
            wdw = vin[14 * KJ0[s] : 14 * KJ0[s] + NW]  # [140, 256]
            vin_p[:, 2 * s, :] = wdw[0:70]
            vin_p[:, 2 * s + 1, :] = wdw[70:140]
        vin_p = np.ascontiguousarray(vin_p)
        par = np.concatenate([wqt_p, wkt_p, kin_p, masks[r], ident], axis=1)
        m = {
            "qpad0": qpad[0],
            "qpad1": qpad[1],
            "par": np.ascontiguousarray(par, dtype=h),
            "vin": vin_p,
        }
        for ci in range(2):
            for co in range(2):
                m[f"wt{ci}{co}"] = wt4[ci][co]
        in_maps.append(m)
    return in_maps


_NC = None


def get_nc():
    global _NC
    if _NC is None:
        _NC = build_nc()
    return _NC


def kernel(q, k, v, conv_w, g_q, g_k, wq, bq, wk, bk):
    in_maps = _host_prep(q, k, v, conv_w, g_q, g_k, wq, bq, wk, bk)
    nc = get_nc()
    res = run_bass_kernel_spmd(nc, in_maps, list(range(NCORES)))
    out = np.empty((B, C, H, W), dtype=np.float32)
    for core in range(NCORES):
        b, r = divmod(core, 4)
        o = res.results[core]["out"]  # [2, 128, NT, 128] f32
        # [co_t, ch, s, y*8+x] -> [256, 16, 64]
        o = o.reshape(2, 128, NT, RSTRIPE, 8).transpose(0, 1, 3, 2, 4).reshape(C, RSTRIPE, 64)
        out[b, :, 16 * r : 16 * r + RSTRIPE, :] = o
    return out
